# revision 12
# baseline (speedup 1.0000x reference)
"""3-layer GATv2 (heads=1, eval) on 8 Trainium2 NeuronCores — Bass/Tile.

kernel(**inputs) takes the FULL inputs (x [100000,128] f32, Wl/Wr [3,128,128],
att [3,128], b [3,128], edge_index [2,1600000] int64) and returns the FULL
[100000, 128] float32 output.

Strategy (graph/data parallel; node-partitioned by dst):
  * core c owns dst nodes [c*12500, (c+1)*12500). Edges grouped on the host
    by (dst block of 128 nodes, src bucket of 25000 rows) with a uniform
    per-(block,bucket) slot budget B1 (multiple of 128); pad slots use idx 0
    and a dloc sentinel (512) whose one-hot row is all-zero downstream.
  * per layer, XL = h@Wl over all N rows (bf16, AllGather'd in 4 chunks so
    communication overlaps the producing block loop) and local XR = h@Wr live
    in DRAM. Only xl[src] is fetched per edge, with SWDGE dma_gather
    (1280-idx gathers per block-pair, round-robin over 4 SWDGE queues for
    concurrent descriptor processing). xr[dst] is expanded on-chip from the
    contiguous 128-row XR block via PE one-hot matmuls.
  * per block: one-hot O[slot,node] built in ONE DVE op (iota==dloc with
    stride-0 broadcast); per 128-slot group, O_T = PE-transpose(O_g) and
    v = O_T.T@XR_blk + I.T@xl_g accumulates in PSUM; z = Prelu(v, 0.2) via
    ACT; e = sum_d z*att (DVE); w = exp(e) (ACT, no segment-max: |e| small
    for this model); Y' = [w*xl | w] (DVE broadcast mult); num/den
    accumulate via matmul(O_g, Y'_g) in PSUM; out = num/(den+1e-16) + bias.
  * next layer's XL/XR rows are produced in the same block pass (PE
    transpose + two matmuls); the XL AllGather chunks fire as quarters of
    the block range complete.
"""

import os
from contextlib import ExitStack

import numpy as np
import ml_dtypes

import concourse.bacc as bacc
import concourse.mybir as mybir
import concourse.tile as tile
from concourse._compat import cdiv
from concourse.masks import make_identity
from concourse.bass_utils import run_bass_kernel_spmd

F32 = mybir.dt.float32
BF16 = mybir.dt.bfloat16
I16 = mybir.dt.int16
AX = mybir.AxisListType
OP = mybir.AluOpType
ACTF = mybir.ActivationFunctionType

D = 128
P = 128
NQ = 4          # SWDGE queues
VG = 4          # v-psum groups per drain tile
AGC = 4         # AllGather chunks


class Cfg:
    def __init__(self, N, cores, b1):
        assert N % cores == 0
        self.N, self.CORES = N, cores
        self.NPC = N // cores
        self.NBLK = cdiv(self.NPC, P)
        self.LASTW = self.NPC - (self.NBLK - 1) * P
        # bucket == AllGather chunk (chunk-major XLf layout): chunk j holds
        # rows {core c, local q in [j*CQ,(j+1)*CQ)} at j*BUCKET + c*CQ + q%CQ
        assert self.NPC % AGC == 0
        self.CQ = self.NPC // AGC
        self.NBUCK = AGC
        self.BUCKET = self.CQ * cores
        assert self.BUCKET - 1 <= 32767
        assert b1 % P == 0
        self.B1 = b1
        self.S1 = b1 // P                 # 128-groups per (block,bucket)
        self.S = self.NBUCK * self.S1     # 128-groups per block
        self.NPAIR = cdiv(self.NBLK, 2)
        # idx columns per (pair,bucket) gather: 2*B1 idx wrapped in 16
        self.GI = 2 * b1 // 16
        self.IDXCOLS = self.NPAIR * self.NBUCK * self.GI


def _wrap16(v):
    L = v.size
    assert L % 16 == 0
    w = v.reshape(L // 16, 16).T.astype(np.int16)
    return np.tile(w, (8, 1))


def host_prep(cfg, edge_index):
    src = np.asarray(edge_index[0], dtype=np.int64)
    dst = np.asarray(edge_index[1], dtype=np.int64)
    # chunk-major physical row of a src node, and its bucket (= chunk)
    s_c, s_q = src // cfg.NPC, src % cfg.NPC
    src_buck = s_q // cfg.CQ
    src_row = s_c * cfg.CQ + (s_q % cfg.CQ)  # row within bucket
    cores = []
    for c in range(cfg.CORES):
        base = c * cfg.NPC
        m = (dst >= base) & (dst < base + cfg.NPC)
        es, ed, buck = src_row[m], dst[m] - base, src_buck[m]
        blk = ed // P
        order = np.lexsort((es, buck, blk))
        es, ed, blk, buck = es[order], ed[order], blk[order], buck[order]
        key = blk * cfg.NBUCK + buck
        bounds = np.searchsorted(key, np.arange(cfg.NBLK * cfg.NBUCK + 1))
        cnt = np.diff(bounds).reshape(cfg.NBLK, cfg.NBUCK)
        if cnt.max() > cfg.B1:
            raise ValueError(f"bucket overflow: {cnt.max()} > {cfg.B1}")
        idx_slots = np.zeros((cfg.NBLK, cfg.NBUCK, cfg.B1), np.int64)
        dl_slots = np.full((cfg.NBLK, cfg.NBUCK, cfg.B1), 512.0, np.float32)
        for b in range(cfg.NBLK):
            for k in range(cfg.NBUCK):
                i0, i1 = bounds[b * cfg.NBUCK + k], bounds[b * cfg.NBUCK + k + 1]
                n = i1 - i0
                idx_slots[b, k, :n] = es[i0:i1]
                dl_slots[b, k, :n] = (ed[i0:i1] - b * P).astype(np.float32)
        # one gather per (pair, bucket): blocks 2p,2p+1 concatenated
        cols = []
        for p_ in range(cfg.NPAIR):
            b0 = 2 * p_
            for k in range(cfg.NBUCK):
                if b0 + 1 < cfg.NBLK:
                    v = np.concatenate([idx_slots[b0, k], idx_slots[b0 + 1, k]])
                else:
                    v = np.concatenate(
                        [idx_slots[b0, k], np.zeros(cfg.B1, np.int64)]
                    )
                cols.append(_wrap16(v))
        xl_idx = np.concatenate(cols, axis=1)
        # dloc per block, column order (k, t, c): slot group s=(k,t)
        dl = dl_slots.reshape(cfg.NBLK, cfg.NBUCK, cfg.S1, P)
        dl = dl.transpose(3, 0, 1, 2).reshape(P, cfg.NBLK * cfg.S)
        dloc = np.ascontiguousarray(dl).astype(ml_dtypes.bfloat16)
        cores.append(dict(xl_idx=xl_idx, dloc=dloc))
    return cores


def host_consts(cfg, Wl, Wr, att, b, x):
    Wl = np.asarray(Wl, np.float32)
    Wr = np.asarray(Wr, np.float32)
    att = np.asarray(att, np.float32)
    b = np.asarray(b, np.float32)
    x = np.asarray(x, np.float32)
    wl_all = Wl.reshape(3 * D, D).astype(ml_dtypes.bfloat16)
    wr_all = Wr.reshape(3 * D, D).astype(ml_dtypes.bfloat16)
    att_mat = np.concatenate(
        [np.tile(att[l][None, :], (P, 1)) for l in range(3)], 0
    ).astype(ml_dtypes.bfloat16)
    bias_mat = np.concatenate(
        [np.tile(b[l][None, :], (P, 1)) for l in range(3)], 0
    ).astype(np.float32)
    # iota repeated: col value c%128, for O build [P, S*128]
    iota = np.tile(np.arange(P, dtype=np.float32)[None, :], (P, cfg.S)).astype(
        ml_dtypes.bfloat16
    )
    out = []
    for c in range(cfg.CORES):
        xT = np.ascontiguousarray(x[c * cfg.NPC : (c + 1) * cfg.NPC].T).astype(
            ml_dtypes.bfloat16
        )
        out.append(
            dict(
                xT_loc=xT,
                Wl_all=wl_all,
                Wr_all=wr_all,
                att_mat=att_mat,
                bias_mat=bias_mat,
                iota_mat=iota,
            )
        )
    return out


def build_program(cfg):
    nc = bacc.Bacc(
        "TRN2", target_bir_lowering=False, debug=False,
        num_devices=cfg.CORES, num_swdge_queues=NQ,
    )
    NPC, NBLK, NBUCK, B1, S, S1 = (
        cfg.NPC, cfg.NBLK, cfg.NBUCK, cfg.B1, cfg.S, cfg.S1,
    )
    NPAIR, GI = cfg.NPAIR, cfg.GI

    xT_loc = nc.dram_tensor("xT_loc", [P, NPC], BF16, kind="ExternalInput")
    Wl_all = nc.dram_tensor("Wl_all", [3 * D, D], BF16, kind="ExternalInput")
    Wr_all = nc.dram_tensor("Wr_all", [3 * D, D], BF16, kind="ExternalInput")
    att_mat = nc.dram_tensor("att_mat", [3 * P, D], BF16, kind="ExternalInput")
    bias_mat = nc.dram_tensor("bias_mat", [3 * P, D], F32, kind="ExternalInput")
    iota_mat = nc.dram_tensor("iota_mat", [P, S * P], BF16, kind="ExternalInput")
    xl_idx = nc.dram_tensor("xl_idx", [P, cfg.IDXCOLS], I16, kind="ExternalInput")
    dloc_in = nc.dram_tensor("dloc", [P, NBLK * S], BF16, kind="ExternalInput")
    out_loc = nc.dram_tensor("out_loc", [NPC, D], F32, kind="ExternalOutput")

    XLb = [nc.dram_tensor(f"XLb{l}", [NPC, D], BF16) for l in range(3)]
    XR = [nc.dram_tensor(f"XR{l}", [NPC, D], BF16) for l in range(3)]
    XLf = [nc.dram_tensor(f"XLf{l}", [cfg.N, D], BF16) for l in range(3)]

    # AllGather chunk j gathers local rows [j*CQ,(j+1)*CQ) of every core into
    # the contiguous chunk-major slice [j*BUCKET,(j+1)*BUCKET) of XLf.
    CQ = cfg.CQ
    cb = [cdiv((j + 1) * CQ, P) for j in range(AGC)]  # producer block idx + 1

    def ag_chunk(l, j):
        ins = XLb[l][j * CQ : (j + 1) * CQ, :].opt()
        outs = XLf[l][j * cfg.BUCKET : (j + 1) * cfg.BUCKET, :].opt()
        nc.gpsimd.collective_compute(
            "AllGather", OP.bypass,
            replica_groups=[list(range(cfg.CORES))],
            ins=[ins], outs=[outs],
        )

    with tile.TileContext(nc) as tc, ExitStack() as ctx:
        consts = ctx.enter_context(tc.tile_pool(name="consts", bufs=1))
        gpool = ctx.enter_context(tc.tile_pool(name="gath", bufs=3))
        opool = ctx.enter_context(tc.tile_pool(name="opool", bufs=2))
        otp = ctx.enter_context(tc.tile_pool(name="otp", bufs=3))
        zpool = ctx.enter_context(tc.tile_pool(name="zpool", bufs=2))
        wrk = ctx.enter_context(tc.tile_pool(name="wrk", bufs=2))
        small = ctx.enter_context(tc.tile_pool(name="small", bufs=3))
        xrp = ctx.enter_context(tc.tile_pool(name="xrp", bufs=3))
        psV = ctx.enter_context(tc.tile_pool(name="psV", bufs=2, space="PSUM"))
        psO = ctx.enter_context(tc.tile_pool(name="psO", bufs=1, space="PSUM"))
        psA = ctx.enter_context(tc.tile_pool(name="psA", bufs=2, space="PSUM"))
        psF = ctx.enter_context(tc.tile_pool(name="psF", bufs=1, space="PSUM"))

        iota_t = consts.tile([P, S * P], BF16, tag="iota")
        nc.sync.dma_start(iota_t[:], iota_mat[:, :])
        dlt = consts.tile([P, NBLK * S], BF16, tag="dlt")
        nc.sync.dma_start(dlt[:], dloc_in[:, :])
        idxt = consts.tile([P, cfg.IDXCOLS], I16, tag="idx")
        nc.sync.dma_start(idxt[:], xl_idx[:, :])
        ident_t = consts.tile([P, P], BF16, tag="ident")
        make_identity(nc, ident_t[:])
        wl_t, wr_t, at_t, bi_t = [], [], [], []
        for l in range(3):
            w1 = consts.tile([P, D], BF16, tag=f"wl{l}")
            nc.sync.dma_start(w1[:], Wl_all[l * D : (l + 1) * D, :])
            w2 = consts.tile([P, D], BF16, tag=f"wr{l}")
            nc.sync.dma_start(w2[:], Wr_all[l * D : (l + 1) * D, :])
            a1 = consts.tile([P, D], BF16, tag=f"att{l}")
            nc.sync.dma_start(a1[:], att_mat[l * P : (l + 1) * P, :])
            b1t = consts.tile([P, D], F32, tag=f"bias{l}")
            nc.sync.dma_start(b1t[:], bias_mat[l * P : (l + 1) * P, :])
            wl_t.append(w1); wr_t.append(w2); at_t.append(a1); bi_t.append(b1t)

        # ---- layer-0 XL/XR production + chunked AllGather ----
        nag = 0
        for cblk in range(NBLK):
            cw = P if cblk < NBLK - 1 else cfg.LASTW
            xTs = wrk.tile([P, P], BF16, tag="xTs")
            nc.sync.dma_start(xTs[:, :cw], xT_loc[:, cblk * P : cblk * P + cw])
            pxl = psF.tile([P, D], F32, tag="fin")
            nc.tensor.matmul(pxl[:cw, :], xTs[:, :cw], wl_t[0][:], start=True, stop=True)
            sxl = small.tile([P, D], BF16, tag="sxl")
            nc.scalar.activation(sxl[:cw, :], pxl[:cw, :], ACTF.Copy)
            nc.sync.dma_start(XLb[0][cblk * P : cblk * P + cw, :], sxl[:cw, :])
            pxr = psF.tile([P, D], F32, tag="fin")
            nc.tensor.matmul(pxr[:cw, :], xTs[:, :cw], wr_t[0][:], start=True, stop=True)
            sxr = small.tile([P, D], BF16, tag="sxr")
            nc.scalar.activation(sxr[:cw, :], pxr[:cw, :], ACTF.Copy)
            nc.sync.dma_start(XR[0][cblk * P : cblk * P + cw, :], sxr[:cw, :])
            if nag < AGC and cblk == cb[nag] - 1:
                ag_chunk(0, nag)
                nag += 1

        for l in range(3):
            nag = 0
            for b in range(NBLK):
                bw = P if b < NBLK - 1 else cfg.LASTW
                pr, bb = divmod(b, 2)
                # -- gathers for this block's pair (issued once per pair) --
                if bb == 0:
                    xlg = gpool.tile([P, NBUCK * 2 * B1], BF16, tag="xlg")
                    for k in range(NBUCK):
                        kb = k * cfg.BUCKET
                        ke = min(kb + cfg.BUCKET, cfg.N)
                        ic0 = (pr * NBUCK + k) * GI
                        nc.gpsimd.dma_gather(
                            xlg[:, k * 2 * B1 : (k + 1) * 2 * B1].rearrange(
                                "p (m x) -> p m x", x=D
                            ),
                            XLf[l][kb:ke, :],
                            idxt[:, ic0 : ic0 + GI],
                            2 * B1, 2 * B1, D,
                            single_packet=False, queue_num=k % NQ,
                        )
                # per-block view of the gathered slots: [P, k, m=(bb,t), c]
                xlb4 = xlg[:].rearrange(
                    "p (k m c) -> p k m c", k=NBUCK, c=P
                )

                # -- XR block (local rows) --
                xrb = xrp.tile([P, D], BF16, tag="xrb")
                if bw < P:
                    nc.vector.memset(xrb[:], 0.0)
                nc.sync.dma_start(xrb[:bw, :], XR[l][b * P : b * P + bw, :])

                # -- O build: one DVE op for the whole block --
                O = opool.tile([P, S * P], BF16, tag="O")
                nc.vector.tensor_tensor(
                    O[:].rearrange("p (s c) -> p s c", c=P),
                    iota_t[:].rearrange("p (s c) -> p s c", c=P),
                    dlt[:, b * S : (b + 1) * S].unsqueeze(2).to_broadcast(
                        [P, S, P]
                    ),
                    op=OP.is_equal,
                )

                # -- v = O_T.T @ XR_blk + I.T @ xl, in PSUM; z = Prelu --
                z = zpool.tile([P, S * P], BF16, tag="z")
                for v0 in range(0, S, VG):
                    vn = min(VG, S - v0)
                    pot = psO.tile([P, VG * P], BF16, tag="ot")
                    sot = otp.tile([P, VG * P], BF16, tag="sot")
                    pv = psV.tile([P, VG * P], F32, tag="v")
                    for gi in range(vn):
                        g = v0 + gi
                        nc.tensor.transpose(
                            pot[:, gi * P : (gi + 1) * P],
                            O[:, g * P : (g + 1) * P], ident_t[:],
                        )
                    nc.scalar.activation(
                        sot[:, : vn * P], pot[:, : vn * P], ACTF.Copy
                    )
                    for gi in range(vn):
                        g = v0 + gi
                        k, t = divmod(g, S1)
                        nc.tensor.matmul(
                            pv[:, gi * P : (gi + 1) * P],
                            sot[:, gi * P : (gi + 1) * P], xrb[:],
                            start=True, stop=False,
                        )
                        nc.tensor.matmul(
                            pv[:, gi * P : (gi + 1) * P],
                            ident_t[:], xlb4[:, k, bb * S1 + t, :],
                            start=False, stop=True,
                        )
                    nc.scalar.activation(
                        z[:, v0 * P : v0 * P + vn * P], pv[:, : vn * P],
                        ACTF.Prelu, alpha=0.2,
                    )

                # -- scores --
                t = wrk.tile([P, S * P], BF16, tag="t")
                nc.vector.tensor_tensor(
                    t[:].rearrange("p (s x) -> p s x", x=D),
                    z[:].rearrange("p (s x) -> p s x", x=D),
                    at_t[l][:].unsqueeze(1).to_broadcast([P, S, D]),
                    op=OP.mult,
                )
                sc = small.tile([P, S], F32, tag="sc")
                nc.vector.tensor_reduce(
                    sc[:], t[:].rearrange("p (s x) -> p s x", x=D),
                    axis=AX.X, op=OP.add,
                )
                w = small.tile([P, S], F32, tag="w")
                nc.scalar.activation(w[:], sc[:], ACTF.Exp)

                # -- Y' = [w*xl | w | pad], num/den matmuls --
                Yp = wrk.tile([P, S * 132], BF16, tag="Yp")
                for k in range(NBUCK):
                    nc.vector.tensor_tensor(
                        Yp[:].rearrange("p (s c) -> p s c", c=132)[
                            :, k * S1 : (k + 1) * S1, :D
                        ],
                        xlb4[:, k, bb * S1 : (bb + 1) * S1, :],
                        w[:, k * S1 : (k + 1) * S1].unsqueeze(2).to_broadcast(
                            [P, S1, D]
                        ),
                        op=OP.mult,
                    )
                nc.vector.tensor_scalar(
                    Yp[:].rearrange("p (s c) -> p s c", c=132)[:, :, D : D + 1],
                    w[:].unsqueeze(2), 1.0, None, op0=OP.mult,
                )
                pa = psA.tile([P, 132], F32, tag="pa")
                for g in range(S):
                    nc.tensor.matmul(
                        pa[:], O[:, g * P : (g + 1) * P],
                        Yp[:, g * 132 : (g + 1) * 132],
                        start=(g == 0), stop=(g == S - 1),
                    )

                # -- output stage --
                den = small.tile([P, 1], F32, tag="den")
                nc.vector.tensor_scalar(
                    den[:], pa[:, D : D + 1], 1e-16, None, op0=OP.add
                )
                rec = small.tile([P, 1], F32, tag="rec")
                nc.vector.reciprocal(rec[:], den[:])
                onum = small.tile([P, D], F32, tag="onum")
                nc.vector.tensor_scalar(
                    onum[:], pa[:, :D], rec[:], None, op0=OP.mult
                )
                nc.vector.tensor_tensor(onum[:], onum[:], bi_t[l][:], op=OP.add)
                if l == 2:
                    nc.sync.dma_start(out_loc[b * P : b * P + bw, :], onum[:bw, :])
                else:
                    hrow = small.tile([P, D], BF16, tag="hrow")
                    nc.scalar.activation(hrow[:], onum[:], ACTF.Relu)
                    pst = psF.tile([P, P], BF16, tag="fint")
                    nc.tensor.transpose(pst[:], hrow[:], ident_t[:])
                    hT = small.tile([P, P], BF16, tag="hT")
                    nc.scalar.activation(hT[:], pst[:], ACTF.Copy)
                    pxl = psF.tile([P, D], F32, tag="fin")
                    nc.tensor.matmul(
                        pxl[:], hT[:], wl_t[l + 1][:], start=True, stop=True
                    )
                    sxl = small.tile([P, D], BF16, tag="sxl")
                    nc.scalar.activation(sxl[:], pxl[:], ACTF.Copy)
                    nc.sync.dma_start(
                        XLb[l + 1][b * P : b * P + bw, :], sxl[:bw, :]
                    )
                    pxr = psF.tile([P, D], F32, tag="fin")
                    nc.tensor.matmul(
                        pxr[:], hT[:], wr_t[l + 1][:], start=True, stop=True
                    )
                    sxr = small.tile([P, D], BF16, tag="sxr")
                    nc.scalar.activation(sxr[:], pxr[:], ACTF.Copy)
                    nc.sync.dma_start(
                        XR[l + 1][b * P : b * P + bw, :], sxr[:bw, :]
                    )
                    if nag < AGC and b == cb[nag] - 1:
                        ag_chunk(l + 1, nag)
                        nag += 1
    nc.compile()
    return nc


def kernel(x, Wl, Wr, att, b, edge_index):
    x = np.asarray(x, np.float32)
    edge_index = np.asarray(edge_index)
    N = x.shape[0]
    CORES = 8

    src = np.asarray(edge_index[0], np.int64)
    dst = np.asarray(edge_index[1], np.int64)
    npc = N // CORES
    nblk = cdiv(npc, P)
    cq = npc // AGC
    sbuck = (src % npc) // cq
    mx = 0
    for c in range(CORES):
        m = (dst >= c * npc) & (dst < (c + 1) * npc)
        key = ((dst[m] - c * npc) // P) * AGC + sbuck[m]
        mx = max(mx, int(np.bincount(key, minlength=nblk * AGC).max()))
    b1 = max(cdiv(mx, P) * P, P)

    cfg = Cfg(N=N, cores=CORES, b1=b1)
    idx_data = host_prep(cfg, edge_index)
    const_data = host_consts(cfg, Wl, Wr, att, b, x)
    nc = build_program(cfg)
    in_maps = [{**idx_data[c], **const_data[c]} for c in range(CORES)]

    prof_dir = os.environ.get("GAT_PROFILE", "")
    if prof_dir:
        import sys
        sys.path.insert(0, "/root/.axon_site")
        from trn_agent_boot import trn_boot
        hook = trn_boot._ntff_profile_via_ctypes("/opt/axon/libaxon_pjrt.so")
        os.makedirs(prof_dir, exist_ok=True)
        with hook(prof_dir, [0]):
            res = run_bass_kernel_spmd(nc, in_maps, core_ids=list(range(CORES)))
    else:
        res = run_bass_kernel_spmd(nc, in_maps, core_ids=list(range(CORES)))

    out = np.concatenate([r["out_loc"] for r in res.results], axis=0)
    return out.astype(np.float32)


# revision 13
# speedup vs baseline: 3.0547x; 3.0547x over previous
"""3-layer GATv2 (heads=1, eval) on 8 Trainium2 NeuronCores — Bass/Tile.

kernel(**inputs) takes the FULL inputs (x [100000,128] f32, Wl/Wr [3,128,128],
att [3,128], b [3,128], edge_index [2,1600000] int64) and returns the FULL
[100000, 128] float32 output.

Strategy (graph/data parallel; node-partitioned by dst):
  * core c owns dst nodes [c*12500, (c+1)*12500). Edges grouped on the host
    by (dst block of 128 nodes, src bucket of 25000 rows) with a uniform
    per-(block,bucket) slot budget B1 (multiple of 128); pad slots use idx 0
    and a dloc sentinel (512) whose one-hot row is all-zero downstream.
  * per layer, XL = h@Wl over all N rows (bf16, AllGather'd in 4 chunks so
    communication overlaps the producing block loop) and local XR = h@Wr live
    in DRAM. Only xl[src] is fetched per edge, with SWDGE dma_gather
    (1280-idx gathers per block-pair, round-robin over 4 SWDGE queues for
    concurrent descriptor processing). xr[dst] is expanded on-chip from the
    contiguous 128-row XR block via PE one-hot matmuls.
  * per block: one-hot O[slot,node] built in ONE DVE op (iota==dloc with
    stride-0 broadcast); per 128-slot group, O_T = PE-transpose(O_g) and
    v = O_T.T@XR_blk + I.T@xl_g accumulates in PSUM; z = Prelu(v, 0.2) via
    ACT; e = sum_d z*att (DVE); w = exp(e) (ACT, no segment-max: |e| small
    for this model); Y' = [w*xl | w] (DVE broadcast mult); num/den
    accumulate via matmul(O_g, Y'_g) in PSUM; out = num/(den+1e-16) + bias.
  * next layer's XL/XR rows are produced in the same block pass (PE
    transpose + two matmuls); the XL AllGather chunks fire as quarters of
    the block range complete.
"""

import os
from contextlib import ExitStack

import numpy as np
import ml_dtypes

import concourse.bacc as bacc
import concourse.mybir as mybir
import concourse.tile as tile
from concourse._compat import cdiv
from concourse.masks import make_identity
from concourse.bass_utils import run_bass_kernel_spmd

F32 = mybir.dt.float32
BF16 = mybir.dt.bfloat16
I16 = mybir.dt.int16
AX = mybir.AxisListType
OP = mybir.AluOpType
ACTF = mybir.ActivationFunctionType

D = 128
P = 128
NQ = 4          # SWDGE queues
VG = 4          # v-psum groups per drain tile
AGC = 4         # AllGather chunks


class Cfg:
    def __init__(self, N, cores, b1):
        assert N % cores == 0
        self.N, self.CORES = N, cores
        self.NPC = N // cores
        self.NBLK = cdiv(self.NPC, P)
        self.LASTW = self.NPC - (self.NBLK - 1) * P
        # bucket == AllGather chunk (chunk-major XLf layout): chunk j holds
        # rows {core c, local q in [j*CQ,(j+1)*CQ)} at j*BUCKET + c*CQ + q%CQ
        assert self.NPC % AGC == 0
        self.CQ = self.NPC // AGC
        self.NBUCK = AGC
        self.BUCKET = self.CQ * cores
        assert self.BUCKET - 1 <= 32767
        assert b1 % P == 0
        self.B1 = b1
        self.S1 = b1 // P                 # 128-groups per (block,bucket)
        self.S = self.NBUCK * self.S1     # 128-groups per block
        self.NPAIR = cdiv(self.NBLK, 2)
        # idx columns per (pair,bucket) gather: 2*B1 idx wrapped in 16
        self.GI = 2 * b1 // 16
        self.IDXCOLS = self.NPAIR * self.NBUCK * self.GI


def _wrap16(v):
    L = v.size
    assert L % 16 == 0
    w = v.reshape(L // 16, 16).T.astype(np.int16)
    return np.tile(w, (8, 1))


def host_prep(cfg, edge_index):
    src = np.asarray(edge_index[0], dtype=np.int64)
    dst = np.asarray(edge_index[1], dtype=np.int64)
    # chunk-major physical row of a src node, and its bucket (= chunk)
    s_c, s_q = src // cfg.NPC, src % cfg.NPC
    src_buck = s_q // cfg.CQ
    src_row = s_c * cfg.CQ + (s_q % cfg.CQ)  # row within bucket
    cores = []
    for c in range(cfg.CORES):
        base = c * cfg.NPC
        m = (dst >= base) & (dst < base + cfg.NPC)
        es, ed, buck = src_row[m], dst[m] - base, src_buck[m]
        blk = ed // P
        order = np.lexsort((es, buck, blk))
        es, ed, blk, buck = es[order], ed[order], blk[order], buck[order]
        key = blk * cfg.NBUCK + buck
        bounds = np.searchsorted(key, np.arange(cfg.NBLK * cfg.NBUCK + 1))
        cnt = np.diff(bounds).reshape(cfg.NBLK, cfg.NBUCK)
        if cnt.max() > cfg.B1:
            raise ValueError(f"bucket overflow: {cnt.max()} > {cfg.B1}")
        idx_slots = np.zeros((cfg.NBLK, cfg.NBUCK, cfg.B1), np.int64)
        dl_slots = np.full((cfg.NBLK, cfg.NBUCK, cfg.B1), 512.0, np.float32)
        for b in range(cfg.NBLK):
            for k in range(cfg.NBUCK):
                i0, i1 = bounds[b * cfg.NBUCK + k], bounds[b * cfg.NBUCK + k + 1]
                n = i1 - i0
                idx_slots[b, k, :n] = es[i0:i1]
                dl_slots[b, k, :n] = (ed[i0:i1] - b * P).astype(np.float32)
        # one gather per (pair, bucket): blocks 2p,2p+1 concatenated
        cols = []
        for p_ in range(cfg.NPAIR):
            b0 = 2 * p_
            for k in range(cfg.NBUCK):
                if b0 + 1 < cfg.NBLK:
                    v = np.concatenate([idx_slots[b0, k], idx_slots[b0 + 1, k]])
                else:
                    v = np.concatenate(
                        [idx_slots[b0, k], np.zeros(cfg.B1, np.int64)]
                    )
                cols.append(_wrap16(v))
        xl_idx = np.concatenate(cols, axis=1)
        # dloc per block, column order (k, t, c): slot group s=(k,t)
        dl = dl_slots.reshape(cfg.NBLK, cfg.NBUCK, cfg.S1, P)
        dl = dl.transpose(3, 0, 1, 2).reshape(P, cfg.NBLK * cfg.S)
        dloc = np.ascontiguousarray(dl).astype(ml_dtypes.bfloat16)
        cores.append(dict(xl_idx=xl_idx, dloc=dloc))
    return cores


def host_consts(cfg, Wl, Wr, att, b, x):
    Wl = np.asarray(Wl, np.float32)
    Wr = np.asarray(Wr, np.float32)
    att = np.asarray(att, np.float32)
    b = np.asarray(b, np.float32)
    x = np.asarray(x, np.float32)
    wl_all = Wl.reshape(3 * D, D).astype(ml_dtypes.bfloat16)
    wr_all = Wr.reshape(3 * D, D).astype(ml_dtypes.bfloat16)
    att_mat = np.concatenate(
        [np.tile(att[l][None, :], (P, 1)) for l in range(3)], 0
    ).astype(ml_dtypes.bfloat16)
    bias_mat = np.concatenate(
        [np.tile(b[l][None, :], (P, 1)) for l in range(3)], 0
    ).astype(np.float32)
    # iota repeated: col value c%128, for O build [P, S*128]
    iota = np.tile(np.arange(P, dtype=np.float32)[None, :], (P, cfg.S)).astype(
        ml_dtypes.bfloat16
    )
    out = []
    for c in range(cfg.CORES):
        xT = np.ascontiguousarray(x[c * cfg.NPC : (c + 1) * cfg.NPC].T).astype(
            ml_dtypes.bfloat16
        )
        out.append(
            dict(
                xT_loc=xT,
                Wl_all=wl_all,
                Wr_all=wr_all,
                att_mat=att_mat,
                bias_mat=bias_mat,
                iota_mat=iota,
            )
        )
    return out


def build_program(cfg):
    nc = bacc.Bacc(
        "TRN2", target_bir_lowering=False, debug=False,
        num_devices=cfg.CORES, num_swdge_queues=NQ,
    )
    NPC, NBLK, NBUCK, B1, S, S1 = (
        cfg.NPC, cfg.NBLK, cfg.NBUCK, cfg.B1, cfg.S, cfg.S1,
    )
    NPAIR, GI = cfg.NPAIR, cfg.GI

    xT_loc = nc.dram_tensor("xT_loc", [P, NPC], BF16, kind="ExternalInput")
    Wl_all = nc.dram_tensor("Wl_all", [3 * D, D], BF16, kind="ExternalInput")
    Wr_all = nc.dram_tensor("Wr_all", [3 * D, D], BF16, kind="ExternalInput")
    att_mat = nc.dram_tensor("att_mat", [3 * P, D], BF16, kind="ExternalInput")
    bias_mat = nc.dram_tensor("bias_mat", [3 * P, D], F32, kind="ExternalInput")
    iota_mat = nc.dram_tensor("iota_mat", [P, S * P], BF16, kind="ExternalInput")
    xl_idx = nc.dram_tensor("xl_idx", [P, cfg.IDXCOLS], I16, kind="ExternalInput")
    dloc_in = nc.dram_tensor("dloc", [P, NBLK * S], BF16, kind="ExternalInput")
    out_loc = nc.dram_tensor("out_loc", [NPC, D], F32, kind="ExternalOutput")

    XLb = [nc.dram_tensor(f"XLb{l}", [NPC, D], BF16) for l in range(3)]
    XR = [nc.dram_tensor(f"XR{l}", [NPC, D], BF16) for l in range(3)]
    XLf = [nc.dram_tensor(f"XLf{l}", [cfg.N, D], BF16) for l in range(3)]

    # AllGather chunk j gathers local rows [j*CQ,(j+1)*CQ) of every core into
    # the contiguous chunk-major slice [j*BUCKET,(j+1)*BUCKET) of XLf.
    CQ = cfg.CQ
    # emit the AG trigger AG_DELAY blocks after the producer block: the
    # trigger occupies GpSimd while waiting, and a late emission point makes
    # the wait already-satisfied so pending gathers aren't head-of-line
    # blocked behind it.
    AG_DELAY = 8
    cb = [min(cdiv((j + 1) * CQ, P) + AG_DELAY, NBLK) for j in range(AGC)]

    def ag_chunk(l, j):
        ins = XLb[l][j * CQ : (j + 1) * CQ, :].opt()
        outs = XLf[l][j * cfg.BUCKET : (j + 1) * cfg.BUCKET, :].opt()
        nc.gpsimd.collective_compute(
            "AllGather", OP.bypass,
            replica_groups=[list(range(cfg.CORES))],
            ins=[ins], outs=[outs],
        )

    with tile.TileContext(nc) as tc, ExitStack() as ctx:
        consts = ctx.enter_context(tc.tile_pool(name="consts", bufs=1))
        gpool = ctx.enter_context(tc.tile_pool(name="gath", bufs=4))
        opool = ctx.enter_context(tc.tile_pool(name="opool", bufs=2))
        otp = ctx.enter_context(tc.tile_pool(name="otp", bufs=3))
        zpool = ctx.enter_context(tc.tile_pool(name="zpool", bufs=2))
        wrk = ctx.enter_context(tc.tile_pool(name="wrk", bufs=2))
        small = ctx.enter_context(tc.tile_pool(name="small", bufs=3))
        xrp = ctx.enter_context(tc.tile_pool(name="xrp", bufs=3))
        psV = ctx.enter_context(tc.tile_pool(name="psV", bufs=2, space="PSUM"))
        psO = ctx.enter_context(tc.tile_pool(name="psO", bufs=2, space="PSUM"))
        psA = ctx.enter_context(tc.tile_pool(name="psA", bufs=2, space="PSUM"))
        psF = ctx.enter_context(tc.tile_pool(name="psF", bufs=1, space="PSUM"))

        iota_t = consts.tile([P, S * P], BF16, tag="iota")
        nc.sync.dma_start(iota_t[:], iota_mat[:, :])
        dlt = consts.tile([P, NBLK * S], BF16, tag="dlt")
        nc.sync.dma_start(dlt[:], dloc_in[:, :])
        idxt = consts.tile([P, cfg.IDXCOLS], I16, tag="idx")
        nc.sync.dma_start(idxt[:], xl_idx[:, :])
        ident_t = consts.tile([P, P], BF16, tag="ident")
        make_identity(nc, ident_t[:])
        wl_t, wr_t, at_t, bi_t = [], [], [], []
        for l in range(3):
            w1 = consts.tile([P, D], BF16, tag=f"wl{l}")
            nc.sync.dma_start(w1[:], Wl_all[l * D : (l + 1) * D, :])
            w2 = consts.tile([P, D], BF16, tag=f"wr{l}")
            nc.sync.dma_start(w2[:], Wr_all[l * D : (l + 1) * D, :])
            a1 = consts.tile([P, D], BF16, tag=f"att{l}")
            nc.sync.dma_start(a1[:], att_mat[l * P : (l + 1) * P, :])
            b1t = consts.tile([P, D], F32, tag=f"bias{l}")
            nc.sync.dma_start(b1t[:], bias_mat[l * P : (l + 1) * P, :])
            wl_t.append(w1); wr_t.append(w2); at_t.append(a1); bi_t.append(b1t)

        # ---- layer-0 XL/XR production + chunked AllGather ----
        nag = 0
        for cblk in range(NBLK):
            cw = P if cblk < NBLK - 1 else cfg.LASTW
            xTs = wrk.tile([P, P], BF16, tag="xTs")
            nc.sync.dma_start(xTs[:, :cw], xT_loc[:, cblk * P : cblk * P + cw])
            pxl = psF.tile([P, D], F32, tag="fin")
            nc.tensor.matmul(pxl[:cw, :], xTs[:, :cw], wl_t[0][:], start=True, stop=True)
            sxl = small.tile([P, D], BF16, tag="sxl")
            nc.scalar.activation(sxl[:cw, :], pxl[:cw, :], ACTF.Copy)
            nc.sync.dma_start(XLb[0][cblk * P : cblk * P + cw, :], sxl[:cw, :])
            pxr = psF.tile([P, D], F32, tag="fin")
            nc.tensor.matmul(pxr[:cw, :], xTs[:, :cw], wr_t[0][:], start=True, stop=True)
            sxr = small.tile([P, D], BF16, tag="sxr")
            nc.scalar.activation(sxr[:cw, :], pxr[:cw, :], ACTF.Copy)
            nc.sync.dma_start(XR[0][cblk * P : cblk * P + cw, :], sxr[:cw, :])
            if nag < AGC and cblk == cb[nag] - 1:
                ag_chunk(0, nag)
                nag += 1

        for l in range(3):
            nag = 0
            for b in range(NBLK):
                bw = P if b < NBLK - 1 else cfg.LASTW
                pr, bb = divmod(b, 2)
                # -- gathers for this block's pair (issued once per pair) --
                if bb == 0:
                    xlg = gpool.tile([P, NBUCK * 2 * B1], BF16, tag="xlg")
                    for k in range(NBUCK):
                        kb = k * cfg.BUCKET
                        ke = min(kb + cfg.BUCKET, cfg.N)
                        ic0 = (pr * NBUCK + k) * GI
                        nc.gpsimd.dma_gather(
                            xlg[:, k * 2 * B1 : (k + 1) * 2 * B1].rearrange(
                                "p (m x) -> p m x", x=D
                            ),
                            XLf[l][kb:ke, :],
                            idxt[:, ic0 : ic0 + GI],
                            2 * B1, 2 * B1, D,
                            single_packet=False, queue_num=k % NQ,
                        )
                # per-block view of the gathered slots: [P, k, m=(bb,t), c]
                xlb4 = xlg[:].rearrange(
                    "p (k m c) -> p k m c", k=NBUCK, c=P
                )

                # -- XR block (local rows) --
                xrb = xrp.tile([P, D], BF16, tag="xrb")
                if bw < P:
                    nc.vector.memset(xrb[:], 0.0)
                nc.sync.dma_start(xrb[:bw, :], XR[l][b * P : b * P + bw, :])

                # -- O build: one DVE op for the whole block --
                O = opool.tile([P, S * P], BF16, tag="O")
                nc.vector.tensor_tensor(
                    O[:].rearrange("p (s c) -> p s c", c=P),
                    iota_t[:].rearrange("p (s c) -> p s c", c=P),
                    dlt[:, b * S : (b + 1) * S].unsqueeze(2).to_broadcast(
                        [P, S, P]
                    ),
                    op=OP.is_equal,
                )

                # -- v = O_T.T @ XR_blk + I.T @ xl, in PSUM; z = Prelu --
                z = zpool.tile([P, S * P], BF16, tag="z")
                for v0 in range(0, S, VG):
                    vn = min(VG, S - v0)
                    pot = psO.tile([P, VG * P], BF16, tag="ot")
                    sot = otp.tile([P, VG * P], BF16, tag="sot")
                    pv = psV.tile([P, VG * P], F32, tag="v")
                    for gi in range(vn):
                        g = v0 + gi
                        nc.tensor.transpose(
                            pot[:, gi * P : (gi + 1) * P],
                            O[:, g * P : (g + 1) * P], ident_t[:],
                        )
                    nc.scalar.activation(
                        sot[:, : vn * P], pot[:, : vn * P], ACTF.Copy
                    )
                    for gi in range(vn):
                        g = v0 + gi
                        k, t = divmod(g, S1)
                        nc.tensor.matmul(
                            pv[:, gi * P : (gi + 1) * P],
                            sot[:, gi * P : (gi + 1) * P], xrb[:],
                            start=True, stop=False,
                        )
                        nc.tensor.matmul(
                            pv[:, gi * P : (gi + 1) * P],
                            ident_t[:], xlb4[:, k, bb * S1 + t, :],
                            start=False, stop=True,
                        )
                    nc.scalar.activation(
                        z[:, v0 * P : v0 * P + vn * P], pv[:, : vn * P],
                        ACTF.Prelu, alpha=0.2,
                    )

                # -- scores --
                t = wrk.tile([P, S * P], BF16, tag="t")
                nc.vector.tensor_tensor(
                    t[:].rearrange("p (s x) -> p s x", x=D),
                    z[:].rearrange("p (s x) -> p s x", x=D),
                    at_t[l][:].unsqueeze(1).to_broadcast([P, S, D]),
                    op=OP.mult,
                )
                sc = small.tile([P, S], F32, tag="sc")
                nc.vector.tensor_reduce(
                    sc[:], t[:].rearrange("p (s x) -> p s x", x=D),
                    axis=AX.X, op=OP.add,
                )
                w = small.tile([P, S], F32, tag="w")
                nc.scalar.activation(w[:], sc[:], ACTF.Exp)

                # -- Y' = [w*xl | w | pad], num/den matmuls --
                Yp = wrk.tile([P, S * 132], BF16, tag="Yp")
                for k in range(NBUCK):
                    nc.vector.tensor_tensor(
                        Yp[:].rearrange("p (s c) -> p s c", c=132)[
                            :, k * S1 : (k + 1) * S1, :D
                        ],
                        xlb4[:, k, bb * S1 : (bb + 1) * S1, :],
                        w[:, k * S1 : (k + 1) * S1].unsqueeze(2).to_broadcast(
                            [P, S1, D]
                        ),
                        op=OP.mult,
                    )
                nc.vector.tensor_scalar(
                    Yp[:].rearrange("p (s c) -> p s c", c=132)[:, :, D : D + 1],
                    w[:].unsqueeze(2), 1.0, None, op0=OP.mult,
                )
                pa = psA.tile([P, 132], F32, tag="pa")
                for g in range(S):
                    nc.tensor.matmul(
                        pa[:], O[:, g * P : (g + 1) * P],
                        Yp[:, g * 132 : (g + 1) * 132],
                        start=(g == 0), stop=(g == S - 1),
                    )

                # -- output stage --
                den = small.tile([P, 1], F32, tag="den")
                nc.vector.tensor_scalar(
                    den[:], pa[:, D : D + 1], 1e-16, None, op0=OP.add
                )
                rec = small.tile([P, 1], F32, tag="rec")
                nc.vector.reciprocal(rec[:], den[:])
                onum = small.tile([P, D], F32, tag="onum")
                nc.vector.tensor_scalar(
                    onum[:], pa[:, :D], rec[:], None, op0=OP.mult
                )
                nc.vector.tensor_tensor(onum[:], onum[:], bi_t[l][:], op=OP.add)
                if l == 2:
                    nc.sync.dma_start(out_loc[b * P : b * P + bw, :], onum[:bw, :])
                else:
                    hrow = small.tile([P, D], BF16, tag="hrow")
                    nc.scalar.activation(hrow[:], onum[:], ACTF.Relu)
                    pst = psF.tile([P, P], BF16, tag="fint")
                    nc.tensor.transpose(pst[:], hrow[:], ident_t[:])
                    hT = small.tile([P, P], BF16, tag="hT")
                    nc.scalar.activation(hT[:], pst[:], ACTF.Copy)
                    pxl = psF.tile([P, D], F32, tag="fin")
                    nc.tensor.matmul(
                        pxl[:], hT[:], wl_t[l + 1][:], start=True, stop=True
                    )
                    sxl = small.tile([P, D], BF16, tag="sxl")
                    nc.scalar.activation(sxl[:], pxl[:], ACTF.Copy)
                    nc.sync.dma_start(
                        XLb[l + 1][b * P : b * P + bw, :], sxl[:bw, :]
                    )
                    pxr = psF.tile([P, D], F32, tag="fin")
                    nc.tensor.matmul(
                        pxr[:], hT[:], wr_t[l + 1][:], start=True, stop=True
                    )
                    sxr = small.tile([P, D], BF16, tag="sxr")
                    nc.scalar.activation(sxr[:], pxr[:], ACTF.Copy)
                    nc.sync.dma_start(
                        XR[l + 1][b * P : b * P + bw, :], sxr[:bw, :]
                    )
                    if nag < AGC and b == cb[nag] - 1:
                        ag_chunk(l + 1, nag)
                        nag += 1
    nc.compile()
    return nc


def kernel(x, Wl, Wr, att, b, edge_index):
    x = np.asarray(x, np.float32)
    edge_index = np.asarray(edge_index)
    N = x.shape[0]
    CORES = 8

    src = np.asarray(edge_index[0], np.int64)
    dst = np.asarray(edge_index[1], np.int64)
    npc = N // CORES
    nblk = cdiv(npc, P)
    cq = npc // AGC
    sbuck = (src % npc) // cq
    mx = 0
    for c in range(CORES):
        m = (dst >= c * npc) & (dst < (c + 1) * npc)
        key = ((dst[m] - c * npc) // P) * AGC + sbuck[m]
        mx = max(mx, int(np.bincount(key, minlength=nblk * AGC).max()))
    b1 = max(cdiv(mx, P) * P, P)

    cfg = Cfg(N=N, cores=CORES, b1=b1)
    idx_data = host_prep(cfg, edge_index)
    const_data = host_consts(cfg, Wl, Wr, att, b, x)
    nc = build_program(cfg)
    in_maps = [{**idx_data[c], **const_data[c]} for c in range(CORES)]

    prof_dir = os.environ.get("GAT_PROFILE", "")
    if prof_dir:
        import sys
        sys.path.insert(0, "/root/.axon_site")
        from trn_agent_boot import trn_boot
        hook = trn_boot._ntff_profile_via_ctypes("/opt/axon/libaxon_pjrt.so")
        os.makedirs(prof_dir, exist_ok=True)
        with hook(prof_dir, [0]):
            res = run_bass_kernel_spmd(nc, in_maps, core_ids=list(range(CORES)))
    else:
        res = run_bass_kernel_spmd(nc, in_maps, core_ids=list(range(CORES)))

    out = np.concatenate([r["out_loc"] for r in res.results], axis=0)
    return out.astype(np.float32)
